# revision 31
# baseline (speedup 1.0000x reference)
"""GATv2 layer kernel for Trainium2 — 8 NeuronCores, SPMD row-sharded.

Math (reference):
    h = x @ W
    s1 = h @ a[:F];  s2 = h @ a[F:]
    e  = leaky_relu(s1[:,None] + s2[None,:], 0.2)
    e  = where(adj > 0, e, -9e15)
    att = softmax(e, axis=1)
    out = elu(att @ h)

Kernel strategy (per core, rows of adj/out sharded across 8 cores; x and
adj columns are rotated per core on the host so each core's own rows are
always chunks 0..SUB-1 — one SPMD program, no separate xs input):
  - s1/s2 are separable: s1 = x @ (W @ a1), s2 = x @ (W @ a2); each core
    computes full h (fp8) from the replicated (rotated) x.
  - exponents are tiny (|s1+s2| <~ 5) so softmax needs no max-subtraction:
    P = adj * exp(lrelu(z)) realized as exp(lrelu(z + adjL)) with
    adjL = (adj-1)*60000 (exp of ~-1.2e4 underflows to exactly 0).
  - the int32->fp16 cast of adj folds BOTH the mask affine and the s1 bias
    in one gpsimd pass: zm = adj*BIG + (s1 - BIG)  (per-partition ptr bias).
  - zm tiles are PE-transposed to [j, i] layout BEFORE the nonlinearity, so
    lrelu(+s2)/exp run in transposed orientation and exp writes the
    attention-matmul rhs (P^T) straight to SBUF in fp8 — no copy pass.
  - attention matmul and the softmax row sums (ones-vector matmul) run as
    fp8 DoubleRow pairs: one matmul contracts two 128-j chunks.
  - emission is a slotted software pipeline (4 slots per j block): each
    slot emits one x-slice / adj pair / cast pair for LATER blocks plus one
    exp-group of the CURRENT block, so every engine's in-order queue
    interleaves pipeline stages at ~1us granularity (avoids head-of-line
    blocking across stages).
  - final: transpose h'^T back, out = elu(h' * 1/rowsum), stream to DRAM.
"""

import sys

if "/opt/trn_rl_repo" not in sys.path:
    sys.path.insert(0, "/opt/trn_rl_repo")

from contextlib import ExitStack

import numpy as np

import concourse.bass as bass
import concourse.tile as tile
from concourse import bacc, mybir
from concourse.masks import make_identity

F32 = mybir.dt.float32
F16 = mybir.dt.float16
F8 = mybir.dt.float8e4
I32 = mybir.dt.int32
AF = mybir.ActivationFunctionType
OP = mybir.AluOpType

N_FULL = 8192
F_IN = 256
F_OUT = 128
N_CORES = 8
NEG_SLOPE = 0.2
MASK_BIG = 60000.0  # exactly representable in fp16; exp(-0.2*60000) == 0


def build_gat(
    n=N_FULL,
    rows=N_FULL // N_CORES,
    f_in=F_IN,
    f_out=F_OUT,
    jb=1024,
    dve_every=2,     # every k-th chunk routes lrelu to DVE instead of ACT
    p_dt=F16,
    PAIR=2,
    adj_bufs=6,
    zm_bufs=20,
    pt_bufs=6,
    ep_bufs=6,
    tq_bufs=4,
    x_bufs=4,
    xt_bufs=4,
):
    """Build the per-core Bass program. All cores run the identical program;
    per-core behavior comes only from per-core input data (adj shard + the
    host-side rotation of x / adj columns). Returns the compiled module."""
    KC = f_in // 128          # k chunks of f_in
    NCH = n // 128            # column chunks of adj / row chunks of h
    SUB = rows // 128         # i subtiles per core
    NJB = n // jb             # j blocks
    CPJ = jb // 128           # 128-chunks per j block
    I_BLK = min(512, rows)
    NIH = rows // I_BLK       # i halves for matmul psum banks
    FO2 = f_out + 2           # h columns + [s1 s2]
    EG = 2                    # chunks per exp group == DoubleRow pair
    SLOTS = CPJ // EG         # pipeline slots per block

    nc = bacc.Bacc(
        "TRN2",
        target_bir_lowering=False,
        debug=False,
        enable_asserts=False,
        num_devices=1,
    )
    x_ap = nc.dram_tensor("x", [n, f_in], F32, kind="ExternalInput").ap()
    w_ap = nc.dram_tensor("w", [f_in, f_out], F32, kind="ExternalInput").ap()
    a_ap = nc.dram_tensor("a", [2 * f_out, 1], F32, kind="ExternalInput").ap()
    adj_ap = nc.dram_tensor("adj", [rows, n], I32, kind="ExternalInput").ap()
    out_ap = nc.dram_tensor("out", [rows, f_out], F32, kind="ExternalOutput").ap()
    rscr = nc.dram_tensor("rscr", [rows], F32, kind="Internal").ap()

    def dram3(ap, off, dims):
        return bass.AP(tensor=ap.tensor, offset=ap.offset + off, ap=dims)

    with tile.TileContext(nc) as tc, ExitStack() as ctx:
        singles = ctx.enter_context(tc.tile_pool(name="singles", bufs=1))

        rhsW = singles.tile([128, KC * FO2], F32)   # per kc: [W chunk | w1 w2]
        ident32 = singles.tile([128, 128], F32)
        make_identity(nc, ident32)
        identp = singles.tile([128, 128], p_dt)
        make_identity(nc, identp)
        h_sb = singles.tile([128, NCH * f_out], p_dt)
        s2st = singles.tile([128, NCH], F32)     # s2[j] in [j%128, j//128]
        s1m = singles.tile([128, SUB], F32)      # s1 - BIG (cast bias ptr)
        ones1 = singles.tile([128, 1], p_dt)
        scratch = singles.tile([128, f_out], F32)
        a1b = singles.tile([128, f_out], F32)
        a2b = singles.tile([128, f_out], F32)
        rstage = singles.tile([128, SUB], F32)
        rinv = singles.tile([128, SUB], F32)

        # ---- constants (highest priority: they unblock the W@a prep) ----
        nc.gpsimd.memset(ones1, 1.0)
        with tc.high_priority(offset=20000):
            nc.scalar.dma_start(a1b, dram3(a_ap, 0, [[0, 128], [1, f_out]]))
            nc.scalar.dma_start(a2b, dram3(a_ap, f_out, [[0, 128], [1, f_out]]))
            for kc in range(KC):
                nc.scalar.dma_start(
                    rhsW[:, kc * FO2 : kc * FO2 + f_out],
                    w_ap[kc * 128 : (kc + 1) * 128, :],
                )
        # w1 = W @ a1, w2 = W @ a2 appended as columns of rhsW
        # (NOTE tensor_tensor_reduce crashes the device — use scalar_tensor_tensor)
        for kc in range(KC):
            for ai, ab in ((0, a1b), (1, a2b)):
                nc.vector.scalar_tensor_tensor(
                    out=scratch,
                    in0=rhsW[:, kc * FO2 : kc * FO2 + f_out],
                    scalar=1.0,
                    in1=ab,
                    op0=OP.mult,
                    op1=OP.mult,
                    accum_out=rhsW[:, kc * FO2 + f_out + ai : kc * FO2 + f_out + ai + 1],
                )
        rhsW16 = singles.tile([128, KC * FO2], p_dt)

        acc_pool = ctx.enter_context(tc.tile_pool(name="acc", bufs=1, space="PSUM"))
        acc_ps = [
            acc_pool.tile([128, I_BLK], F32, name=f"acc{ih}", tag=f"acc{ih}")
            for ih in range(NIH)
        ]
        # both rowsum accumulators share one PSUM bank at partition
        # offsets 0 and 64 (legal matmul tile positions for M=1)
        rs_bank = acc_pool.tile([128, I_BLK], F32, name="rs_bank", tag="rs_bank")
        rs_ps = [rs_bank[64 * ih : 64 * ih + 1, :] for ih in range(NIH)]

        with ExitStack() as bctx:
            xpool = bctx.enter_context(tc.tile_pool(name="xpool", bufs=x_bufs))
            xtp = bctx.enter_context(tc.tile_pool(name="xtp", bufs=xt_bufs))
            pa_ps = bctx.enter_context(tc.tile_pool(name="pa_ps", bufs=1, space="PSUM"))
            tqp = bctx.enter_context(tc.tile_pool(name="tqp", bufs=tq_bufs, space="PSUM"))
            adjp = bctx.enter_context(tc.tile_pool(name="adjp", bufs=adj_bufs))
            zmp = bctx.enter_context(tc.tile_pool(name="zmp", bufs=zm_bufs))
            ptp = bctx.enter_context(tc.tile_pool(name="ptp", bufs=pt_bufs))
            ep = bctx.enter_context(tc.tile_pool(name="ep", bufs=ep_bufs))

            nc.vector.tensor_copy(rhsW16, rhsW)

            def emit_xdma(b):
                xbt = xpool.tile([128, CPJ * f_in], F32, tag="xbt")
                nc.scalar.dma_start(
                    xbt,
                    dram3(
                        x_ap,
                        b * CPJ * 128 * f_in,
                        [[f_in, 128], [128 * f_in, CPJ], [1, f_in]],
                    ),
                )
                return xbt

            def emit_A_slice(xbt, b, pr):
                """Two x chunks: fp32 transposes, one fp16 staging copy,
                h matmuls, h/s2 (and s1-BIG) stashes."""
                own = b == 0
                tp = pa_ps.tile([128, 2 * f_in], F32, tag="pa")
                for cc in range(2):
                    c = 2 * pr + cc
                    for kc in range(KC):
                        nc.tensor.transpose(
                            tp[:, cc * f_in + kc * 128 : cc * f_in + kc * 128 + 128],
                            xbt[:, c * f_in + kc * 128 : c * f_in + (kc + 1) * 128],
                            ident32,
                        )
                xT2 = xtp.tile([128, 2 * f_in], p_dt, tag="xT")
                nc.vector.tensor_copy(xT2, tp)
                hps_full = pa_ps.tile([128, 2 * f_in], F32, tag="pa", name=f"hps_{b}_{pr}")
                hps = hps_full[:, : 2 * FO2]
                for cc in range(2):
                    for kc in range(KC):
                        nc.tensor.matmul(
                            hps[:, cc * FO2 : (cc + 1) * FO2],
                            lhsT=xT2[:, cc * f_in + kc * 128 : cc * f_in + (kc + 1) * 128],
                            rhs=rhsW16[:, kc * FO2 : (kc + 1) * FO2],
                            start=(kc == 0),
                            stop=(kc == KC - 1),
                        )
                ic0 = b * CPJ + 2 * pr
                h2 = hps.rearrange("p (c f) -> p c f", c=2)
                nc.vector.tensor_copy(
                    h_sb[:, ic0 * f_out : (ic0 + 2) * f_out].rearrange(
                        "p (c f) -> p c f", c=2
                    ),
                    h2[:, :, :f_out],
                )
                nc.vector.tensor_copy(
                    s2st[:, ic0 : ic0 + 2].rearrange("p (c f) -> p c f", c=2),
                    h2[:, :, f_out + 1 : f_out + 2],
                )
                if own:
                    nc.vector.tensor_scalar(
                        out=s1m[:, ic0 : ic0 + 2].rearrange("p (c f) -> p c f", c=2),
                        in0=h2[:, :, f_out : f_out + 1],
                        scalar1=-MASK_BIG,
                        scalar2=None,
                        op0=OP.add,
                        op1=OP.bypass,
                    )

            def emit_adj_pair(b, p):
                adj_t = adjp.tile([128, PAIR * jb], I32, tag="adj", name=f"adj_{b}_{p}")
                nc.sync.dma_start(
                    adj_t,
                    dram3(
                        adj_ap,
                        b * jb + p * PAIR * 128 * n,
                        [[n, 128], [128 * n, PAIR], [1, jb]],
                    ),
                )
                return adj_t

            def emit_casts(b, p, adj_t, zm_list):
                for q in range(PAIR):
                    s = PAIR * p + q
                    zm = zmp.tile([128, jb], p_dt, tag="zm", name=f"zm_{b}_{s}")
                    asl = adj_t[:, q * jb : (q + 1) * jb]
                    # block 0 is the startup critical path: spread its casts
                    # across Pool/ACT/DVE so the first exp-group starts early
                    eng = "p" if b > 0 else "paad"[s % 4]
                    if eng == "a":
                        # Prelu with alpha=1 == identity affine with ptr bias
                        nc.scalar.activation(
                            out=zm, in_=asl, func=AF.Prelu,
                            bias=s1m[:, s : s + 1], scale=MASK_BIG, alpha=1.0,
                        )
                    elif eng == "d":
                        nc.vector.tensor_scalar(
                            out=zm, in0=asl, scalar1=MASK_BIG,
                            scalar2=s1m[:, s : s + 1], op0=OP.mult, op1=OP.add,
                        )
                    else:
                        nc.gpsimd.tensor_scalar(
                            out=zm, in0=asl, scalar1=MASK_BIG,
                            scalar2=s1m[:, s : s + 1], op0=OP.mult, op1=OP.add,
                        )
                    zm_list.append(zm)

            ucount = [0]
            mm_pending = []
            DR = mybir.MatmulPerfMode.DoubleRow

            def emit_group_matmuls(g0, pt2):
                pt3 = pt2.rearrange("p (t n) -> p t n", t=2)
                for ih in range(NIH):
                    rsl = pt3[:, :, ih * I_BLK : (ih + 1) * I_BLK]
                    for t in range(2):
                        nc.tensor.matmul(
                            acc_ps[ih],
                            lhsT=h_sb[:, (g0 + t) * f_out : (g0 + t + 1) * f_out],
                            rhs=rsl[:, t, :],
                            start=(g0 == 0 and t == 0),
                            stop=(g0 == NCH - 2 and t == 1),
                            skip_group_check=True,
                        )
                        nc.tensor.matmul(
                            rs_ps[ih],
                            lhsT=ones1,
                            rhs=rsl[:, t, :],
                            start=(g0 == 0 and t == 0),
                            stop=(g0 == NCH - 2 and t == 1),
                            skip_group_check=True,
                        )

            def emit_E_group(b, c0, zm_tiles):
                """transposes -> lrelu -> exp per chunk (exp writes half of a
                pair-wide fp8 tile for DoubleRow), then the (delayed) matmuls
                of the previous group."""
                pt2 = ptp.tile([128, EG * rows], p_dt, tag="pt")
                for c in range(c0, c0 + EG):
                    tq_t = tqp.tile([128, rows], p_dt, tag="tq", name=f"tq_{b}_{c}")
                    for s in range(SUB):
                        nc.tensor.transpose(
                            tq_t[:, s * 128 : (s + 1) * 128],
                            zm_tiles[s][:, c * 128 : (c + 1) * 128],
                            identp,
                        )
                    g = b * CPJ + c
                    s2ptr = s2st[:, g : g + 1]
                    use_dve = dve_every > 0 and (
                        ucount[0] % dve_every == 0
                    )
                    ucount[0] += 1
                    l_t = ep.tile([128, rows], p_dt, tag="l")
                    if use_dve:
                        z_t = ep.tile([128, rows], p_dt, tag="z")
                        nc.vector.tensor_scalar(
                            out=z_t, in0=tq_t, scalar1=s2ptr,
                            scalar2=None, op0=OP.add, op1=OP.bypass,
                        )
                        nc.vector.scalar_tensor_tensor(
                            out=l_t, in0=z_t, scalar=NEG_SLOPE, in1=z_t,
                            op0=OP.mult, op1=OP.max,
                        )
                    else:
                        nc.scalar.activation(
                            out=l_t, in_=tq_t, func=AF.Prelu,
                            bias=s2ptr, scale=1.0, alpha=NEG_SLOPE,
                        )
                    nc.scalar.activation(
                        out=pt2[:, (c - c0) * rows : (c - c0 + 1) * rows],
                        in_=l_t,
                        func=AF.Exp,
                    )
                if mm_pending:
                    emit_group_matmuls(*mm_pending.pop(0))
                mm_pending.append((b * CPJ + c0, pt2))

            # ---- pipeline prologue: x for blocks 0/1, adj for 0/1,
            # A + casts for block 0. Block 0's x and adj are pinned to the
            # front of the scheduler's priority order: the whole pipeline
            # ramp hangs off them ----
            with tc.high_priority(offset=15000):
                xbts = {0: emit_xdma(0)}
            xbts[1] = emit_xdma(1)
            adjs = {0: [], 1: []}
            with tc.high_priority(offset=10000):
                for pr in range(SLOTS):
                    adjs[0].append(emit_adj_pair(0, pr))
            zms = {0: []}
            for pr in range(SLOTS):
                emit_A_slice(xbts[0], 0, pr)
                adjs[1].append(emit_adj_pair(1, pr))
                emit_casts(0, pr, adjs[0][pr], zms[0])

            # ---- slotted main loop: adj DMA runs 2 blocks ahead, A/casts
            # one block ahead, exp-groups current ----
            for b in range(NJB):
                zm_tiles = zms.pop(b)
                if b + 1 < NJB:
                    zms[b + 1] = []
                for slot in range(SLOTS):
                    if slot == 0 and b + 2 < NJB:
                        xbts[b + 2] = emit_xdma(b + 2)
                        adjs[b + 2] = []
                    if b + 2 < NJB:
                        adjs[b + 2].append(emit_adj_pair(b + 2, slot))
                    if b + 1 < NJB:
                        emit_A_slice(xbts[b + 1], b + 1, slot)
                        emit_casts(b + 1, slot, adjs[b + 1][slot], zms[b + 1])
                    emit_E_group(b, slot * EG, zm_tiles)
            while mm_pending:
                emit_group_matmuls(*mm_pending.pop(0))

        # ---- phase C: normalize + elu + store ----
        with ExitStack() as cctx:
            fpool = cctx.enter_context(tc.tile_pool(name="fpool", bufs=4))
            fps = cctx.enter_context(tc.tile_pool(name="fps", bufs=2, space="PSUM"))
            # rowsums -> DRAM -> [i%128, i//128] layout -> 1/r
            r_sb = fpool.tile([1, rows], F32, tag="r_sb", bufs=1)
            for ih in range(NIH):
                nc.vector.tensor_copy(
                    r_sb[:, ih * I_BLK : (ih + 1) * I_BLK], rs_ps[ih]
                )
                nc.scalar.dma_start(
                    dram3(rscr, ih * I_BLK, [[I_BLK, 1], [1, I_BLK]]),
                    r_sb[:, ih * I_BLK : (ih + 1) * I_BLK],
                )
            nc.scalar.dma_start(rstage, dram3(rscr, 0, [[1, 128], [128, SUB]]))
            nc.vector.reciprocal(rinv, rstage)
            hTn = fpool.tile([128, rows], F32, tag="hTn", bufs=1)
            for ih in range(NIH):
                nc.vector.tensor_copy(hTn[:, ih * I_BLK : (ih + 1) * I_BLK], acc_ps[ih])
            for s in range(SUB):
                tb = fps.tile([128, 128], F32)
                nc.tensor.transpose(tb, hTn[:, s * 128 : (s + 1) * 128], ident32)
                # elu(v) with v = h'_unnorm * rinv:  relu(v) + exp(min(v, 0)) - 1
                t1 = fpool.tile([128, f_out], F32)
                nc.vector.tensor_scalar(
                    out=t1, in0=tb, scalar1=rinv[:, s : s + 1], scalar2=0.0,
                    op0=OP.mult, op1=OP.max,
                )
                t2 = fpool.tile([128, f_out], F32)
                nc.vector.tensor_scalar(
                    out=t2, in0=tb, scalar1=rinv[:, s : s + 1], scalar2=0.0,
                    op0=OP.mult, op1=OP.min,
                )
                t3 = fpool.tile([128, f_out], F32)
                nc.scalar.activation(out=t3, in_=t2, func=AF.Exp)
                o_t = fpool.tile([128, f_out], F32)
                nc.vector.scalar_tensor_tensor(
                    out=o_t, in0=t3, scalar=-1.0, in1=t1, op0=OP.add, op1=OP.add
                )
                nc.scalar.dma_start(out_ap[s * 128 : (s + 1) * 128, :], o_t)

    nc.compile()
    return nc


_CACHE = {}


def _compiled_full():
    if "nc" not in _CACHE:
        _CACHE["nc"] = build_gat()
    return _CACHE["nc"]


def make_in_maps(x, W, a, adj):
    rows = N_FULL // N_CORES
    in_maps = []
    for c in range(N_CORES):
        sl = slice(c * rows, (c + 1) * rows)
        in_maps.append(
            {
                "x": np.ascontiguousarray(np.roll(x, -c * rows, axis=0)),
                "w": W,
                "a": a,
                "adj": np.ascontiguousarray(np.roll(adj[sl], -c * rows, axis=1)),
            }
        )
    return in_maps


def kernel(x, W, a, adj):
    from concourse.bass_utils import run_bass_kernel_spmd

    nc = _compiled_full()
    x = np.ascontiguousarray(np.asarray(x, dtype=np.float32))
    W = np.ascontiguousarray(np.asarray(W, dtype=np.float32))
    a = np.ascontiguousarray(np.asarray(a, dtype=np.float32))
    adj = np.asarray(adj)
    assert adj.dtype == np.int32
    in_maps = make_in_maps(x, W, a, adj)
    res = run_bass_kernel_spmd(nc, in_maps, core_ids=list(range(N_CORES)))
    out = np.concatenate([res.results[c]["out"] for c in range(N_CORES)], axis=0)
    return out.astype(np.float32)
